# revision 1
# baseline (speedup 1.0000x reference)
"""BandSplitLinear Trainium2 kernel (host-transposed fp16 I/O, pure matmul).

Strategy (per core, batch-parallel over 8 cores):
  - Fold w_pre @ w_post into one 128x128 matrix per band on the host (no
    nonlinearity between the linears); biases applied host-side.
  - Carve the frequency axis into 33 aligned segments of 32 bins (grid
    f + 22 = 32*j + u). Every band spans <= 2 adjacent segments, so the
    folded weights form a block-TRIDIAGONAL [33x33] structure of 128-col
    blocks (63 nonzero) over the feature layout g = u*4 + c; the "lower"
    off-diagonal blocks are column-trimmed to the straddling band's tail
    (their matmuls accumulate into the partition prefix [0, 4*uhi)).
  - Host passes x already cast to fp16, zero-padded to the segment grid,
    and permuted to [g = u*4+c (128), j(33), T] — exactly the SBUF layout,
    so loads/stores are plain partition-range strided DMAs.
    On-chip data flow is pure: HBM->SBUF loads, fp16 matmuls with fp32
    PSUM accumulation, PSUM->SBUF cast copies, SBUF->HBM stores in the
    same layout. Zero on-chip transposes, gather/scatter, or packing.
    Host permutes/casts the output back to [C, T, F] fp32.
  - DMA-bound (~19.6 MB/core; reads ~14 B/ns per DMA engine, writes ~24,
    16 engines; one dma_start binds to one engine). Traffic is split into
    ~100 DMAs across the three trigger-engine FIFO queues (SP/Act HWDGE +
    Pool SWDGE) so engines stay saturated; the PE-critical prefix (weight
    ranges + all 4 c-planes per group) is interleaved across queues; a
    dummy 1-col matmul gates PE start until load group 2 is resident so
    the matmul wave runs warm and contiguous; the final store groups are
    u-split so the drain uses many engines.
"""

import numpy as np

import concourse.tile as tile
from concourse import bacc, mybir
from concourse.bass_utils import run_bass_kernel_spmd


# ---- problem constants (hardcoded per spec) ----
B, C, T, F = 8, 4, 1000, 1025
N_CORES = 8
SEG = 32
FOFF = 22  # grid phase: f + FOFF = 32*j + u; band starts align for f >= 490
NSEG = (F - 1 + FOFF) // SEG + 1  # 33
FPAD = NSEG * SEG  # 1056 padded f rows
P = 128
T_CHUNKS = [(0, 512), (512, 488)]
LOAD_GROUPS = [2, 3, 4, 4, 5, 5, 5, 5]  # j-segments per load group (sum 33)
LOAD_EMIT_ORDER = [0, 1, 2, 3, 4, 5, 6, 7]
PE_GATE_GROUP = 2  # first matmul waits for this load group: warm contiguous run
STORE_GROUPS = [4, 4, 5, 5, 5, 4, 3, 2, 1]  # j-segments per store group (sum 33)

_F32 = mybir.dt.float32
_F16 = mybir.dt.float16


def _build_bands():
    f, interval = 0, 4
    groups = []
    while f < F:
        end = min(f + interval, F)
        groups.append((f, end))
        f = end
        if interval < 32:
            interval += 1
    return groups  # list of (start, end), disjoint, covering [0, F)


def _block_structure():
    """Nonzero (j_out, j_in) block pairs, grouped by j_out (ascending j_in)."""
    bands = _build_bands()
    pairs = set()
    for start, end in bands:
        segs = set(range((start + FOFF) // SEG, (end - 1 + FOFF) // SEG + 1))
        for ji in segs:
            for jo in segs:
                pairs.add((jo, ji))
    jin_lists = [sorted(ji for (jo, ji) in pairs if jo == j) for j in range(NSEG)]
    return bands, jin_lists


def _build_weight_blocks(w_pre, w_post):
    """Host: fold per-band linears and scatter into segment-pair blocks."""
    bands, jin_lists = _block_structure()
    wc = np.einsum(
        "kio,kod->kid", w_pre.astype(np.float64), w_post.astype(np.float64)
    )  # [45, 128, 128], both feature dims indexed by w*4 + c
    blocks = {}
    for k, (start, end) in enumerate(bands):
        fs = np.arange(start, end)
        js = (fs + FOFF) // SEG
        us = (fs + FOFF) % SEG
        for ji in np.unique(js):
            for jo in np.unique(js):
                key = (int(jo), int(ji))
                if key not in blocks:
                    blocks[key] = np.zeros((P, P), dtype=np.float64)
                blk = blocks[key]
                mi = js == ji
                mo = js == jo
                wi = fs[mi] - start
                wo = fs[mo] - start
                for ci in range(C):
                    for co in range(C):
                        # feature layout g = u*4 + c (u-major, c interleaved)
                        blk[np.ix_(us[mi] * C + ci, us[mo] * C + co)] = wc[k][
                            np.ix_(wi * C + ci, wo * C + co)
                        ]
    # Per jo, order blocks [diag, upper (ji=jo+1), lower (ji=jo-1)]. The
    # lower block's nonzero output columns are only g < 4*uhi (the
    # straddling band's tail at the bottom of seg jo) -- trim them: that
    # matmul then writes just partitions [0, 4*uhi), accumulating after the
    # full-width diag/upper matmuls. Saves wall bytes; numerically
    # identical (trimmed columns are exact zeros).
    cols = []
    block_meta = []  # per jo: list of (ji, col_off, ncols)
    off = 0
    for jo in range(NSEG):
        metas = []
        for ji in [jo] + [j for j in (jo + 1, jo - 1) if j in jin_lists[jo]]:
            blk = blocks[(jo, ji)]
            if ji == jo - 1:
                fb = SEG * jo - FOFF
                (uhi,) = [e - fb for (s, e) in bands if s < fb < e]
                blk = blk[:, : C * uhi]
            metas.append((ji, off, blk.shape[1]))
            cols.append(blk)
            off += blk.shape[1]
        block_meta.append(metas)
    wall = np.concatenate(cols, axis=1).astype(np.float16)  # [128, total]
    return wall, block_meta, jin_lists


def _bias_field(bands, b_pre, w_post, b_post):
    """bias[c, f]: the constant added to out[., c, ., f]."""
    bc = (
        np.einsum("ko,kod->kd", b_pre.astype(np.float64), w_post.astype(np.float64))
        + b_post.astype(np.float64)
    )
    field = np.zeros((C, F), dtype=np.float64)
    for k, (start, end) in enumerate(bands):
        for c in range(C):
            field[c, start:end] = bc[k, (np.arange(end - start)) * C + c]
    return field.astype(np.float32)


def _build_nc(block_meta, total_cols):
    nc = bacc.Bacc("TRN2", target_bir_lowering=False, debug=False)
    xs = nc.dram_tensor("xs", [P, NSEG, T], _F16, kind="ExternalInput")
    wall = nc.dram_tensor("wall", [P, total_cols], _F16, kind="ExternalInput")
    ys = nc.dram_tensor("ys", [P, NSEG, T], _F16, kind="ExternalOutput")

    # wall load split points (block indices -> column offsets): small first
    # ranges so early jo can start
    flat = [m for metas in block_meta for m in metas]
    nblk = len(flat)
    block_coffs = [m[1] for m in flat] + [total_cols]
    wall_splits = [0, 2, 6, 13, 25]
    while wall_splits[-1] < nblk:
        wall_splits.append(min(wall_splits[-1] + 18, nblk))

    with tile.TileContext(nc) as tc:
        import contextlib

        ctx = contextlib.ExitStack()
        with ctx:
            wall_pool = ctx.enter_context(tc.tile_pool(name="wall", bufs=1))
            at_pools = [
                ctx.enter_context(tc.tile_pool(name=f"atg{i}", bufs=1))
                for i in range(len(LOAD_GROUPS))
            ]
            y_pools = [
                ctx.enter_context(tc.tile_pool(name=f"yg{i}", bufs=1))
                for i in range(len(STORE_GROUPS))
            ]
            psy_pool = ctx.enter_context(
                tc.tile_pool(name="psy", bufs=8, space="PSUM")
            )

            # ---- input loads: [g = c*32+u, j*T + t] per group ----
            at_tiles = []  # (j0, tile) per group
            j0 = 0
            for gi, gn in enumerate(LOAD_GROUPS):
                at_tiles.append((j0, at_pools[gi].tile([P, gn * T], _F16, name=f"atg{gi}")))
                j0 += gn

            wall_sb = wall_pool.tile([P, total_cols], _F16)
            wall_ranges = [
                (block_coffs[lo], block_coffs[hi])
                for lo, hi in zip(wall_splits, wall_splits[1:])
            ]
            # valid partitions: j=0 has g >= 4*FOFF, j=32 has g < 4*UTAIL
            UTAIL = F - (SEG * (NSEG - 1) - FOFF)  # 23 valid u rows in j=32

            # Each trigger engine owns one FIFO DMA queue; order within a
            # queue approximates completion order. The PE-critical prefix
            # (weight ranges + all 4 partition-quarters of each at group)
            # is spread over all three queues: sync carries quarters 0,1;
            # scalar q2 + even wall ranges; gpsimd q3 + odd wall ranges.
            # Loads are split by t-chunk to keep many mid-size DMAs in
            # flight.
            def emit_wall_range(idx):
                if idx < len(wall_ranges):
                    lo, hi = wall_ranges[idx]
                    eng = nc.scalar if idx % 2 == 0 else nc.gpsimd
                    eng.dma_start(wall_sb[:, lo:hi], wall.ap()[:, lo:hi])

            emit_wall_range(0)
            emit_wall_range(1)
            for ei, gi in enumerate(LOAD_EMIT_ORDER):
                gn = LOAD_GROUPS[gi]
                j0, at_t = at_tiles[gi]
                for t0, tn in T_CHUNKS:
                    for q in range(4):
                        eng = [nc.sync, nc.sync, nc.scalar, nc.gpsimd][q]
                        dst = at_t[q * SEG : (q + 1) * SEG, :].rearrange(
                            "p (j t) -> p j t", j=gn
                        )[:, :, t0 : t0 + tn]
                        eng.dma_start(
                            dst,
                            xs.ap()[
                                q * SEG : (q + 1) * SEG, j0 : j0 + gn, t0 : t0 + tn
                            ],
                        )
                emit_wall_range(ei + 2)
            for idx in range(len(LOAD_GROUPS) + 2, len(wall_ranges)):
                emit_wall_range(idx)

            def at_slice(ji, t0, tn):
                for gi, gn in enumerate(LOAD_GROUPS):
                    j0, at_t = at_tiles[gi]
                    if j0 <= ji < j0 + gn:
                        return at_t[:, (ji - j0) * T + t0 : (ji - j0) * T + t0 + tn]
                raise AssertionError(ji)

            # ---- y staging tiles per store group ----
            y_tiles = []
            j0 = 0
            for gi, gn in enumerate(STORE_GROUPS):
                y_tiles.append((j0, y_pools[gi].tile([P, gn * T], _F16, name=f"yg{gi}")))
                j0 += gn

            # ---- matmul wavefront over jo, PSUM -> y copies, group stores ----
            # PE gate: a 1-column dummy matmul on a later load group delays
            # PE's in-order stream until enough input is resident, so the
            # real matmuls run back-to-back at full (warm) clock instead of
            # chasing the load wavefront through p-state resets.
            gate_ps = psy_pool.tile([P, 512], _F32, name="psy")
            nc.tensor.matmul(
                gate_ps[:, 0:1],
                lhsT=wall_sb[:, 0:P],
                rhs=at_tiles[PE_GATE_GROUP][1][:, 0:1],
                start=True,
                stop=True,
            )

            gi_store = 0
            for jo in range(NSEG):
                metas = block_meta[jo]
                nw = len(metas)
                yj0, y_t = y_tiles[gi_store]
                for t0, tn in T_CHUNKS:
                    psy = psy_pool.tile([P, 512], _F32, name="psy")
                    for i, (ji, coff, ncols) in enumerate(metas):
                        # trimmed lower blocks write only partitions
                        # [0, ncols), accumulating onto the full-width result
                        nc.tensor.matmul(
                            psy[0:ncols, 0:tn],
                            lhsT=wall_sb[:, coff : coff + ncols],
                            rhs=at_slice(ji, t0, tn),
                            start=(i == 0),
                            stop=(i == nw - 1),
                        )
                    dst = y_t[:, (jo - yj0) * T + t0 : (jo - yj0) * T + t0 + tn]
                    if jo % 2 == 0:
                        nc.scalar.copy(dst, psy[:, 0:tn])
                    else:
                        nc.vector.tensor_copy(dst, psy[:, 0:tn])

                # group finished -> store it; partition-quarters split across
                # all three DMA queues so the drain runs multi-queue. Pad
                # partitions of the first/last segment are never written;
                # host ignores them.
                gn = STORE_GROUPS[gi_store]
                if jo == yj0 + gn - 1:
                    first = gi_store == 0
                    # split the final two groups by partition-quarters: the
                    # tail then drains on many DMA engines instead of one
                    last = gi_store == len(STORE_GROUPS) - 1
                    src = y_t.rearrange("p (j t) -> p j t", j=gn)
                    if first:
                        # j=0: only g >= 4*FOFF valid
                        nc.gpsimd.dma_start(
                            ys.ap()[C * FOFF :, 0, :], y_t[C * FOFF :, 0:T]
                        )
                        for q, eng in enumerate(
                            [nc.scalar, nc.sync, nc.gpsimd, nc.scalar]
                        ):
                            eng.dma_start(
                                ys.ap()[q * SEG : (q + 1) * SEG, 1:gn, :],
                                src[q * SEG : (q + 1) * SEG, 1:gn, :],
                            )
                    elif last:
                        # j=32 only: valid g < 4*UTAIL = 92
                        for (p0, p1), eng in zip(
                            [(0, 32), (32, 64), (64, C * UTAIL)],
                            [nc.sync, nc.scalar, nc.gpsimd],
                        ):
                            eng.dma_start(
                                ys.ap()[p0:p1, NSEG - 1, :],
                                y_t[p0:p1, (gn - 1) * T : gn * T],
                            )
                        if gn > 1:
                            for q, eng in enumerate(
                                [nc.sync, nc.scalar, nc.gpsimd, nc.sync]
                            ):
                                eng.dma_start(
                                    ys.ap()[
                                        q * SEG : (q + 1) * SEG,
                                        yj0 : yj0 + gn - 1,
                                        :,
                                    ],
                                    src[q * SEG : (q + 1) * SEG, 0 : gn - 1, :],
                                )
                    else:
                        for q, eng in enumerate(
                            [
                                nc.scalar,
                                nc.sync,
                                nc.gpsimd,
                                nc.scalar if gi_store % 2 == 0 else nc.gpsimd,
                            ]
                        ):
                            eng.dma_start(
                                ys.ap()[q * SEG : (q + 1) * SEG, yj0 : yj0 + gn, :],
                                src[q * SEG : (q + 1) * SEG, :, :],
                            )
                    gi_store += 1
    nc.compile()
    return nc


_CACHE = {}


def build_in_maps(x, wall):
    """Host prep: wall is already the flat [g_in, packed cols] matrix; x is
    cast fp16, padded to the 1056-row segment grid, permuted to
    [g = u*4+c (128), j(33), T] so each SBUF partition reads one contiguous
    DRAM run."""
    wall2 = np.ascontiguousarray(wall)
    xp = np.zeros((B, C, FPAD, T), dtype=np.float16)
    xp[:, :, FOFF : FOFF + F, :] = np.asarray(x, np.float32).astype(
        np.float16
    ).transpose(0, 1, 3, 2)
    xp = np.ascontiguousarray(
        xp.reshape(B, C, NSEG, SEG, T)
        .transpose(0, 3, 1, 2, 4)  # [B, u, c, j, T]
        .reshape(B, P, NSEG, T)
    )
    return [{"xs": xp[b], "wall": wall2} for b in range(N_CORES)]


def kernel(x, w_pre, b_pre, w_post, b_post):
    x = np.asarray(x, dtype=np.float32)
    w_pre = np.asarray(w_pre, dtype=np.float32)
    b_pre = np.asarray(b_pre, dtype=np.float32)
    w_post = np.asarray(w_post, dtype=np.float32)
    b_post = np.asarray(b_post, dtype=np.float32)

    bands, _ = _block_structure()
    wall, block_meta, _ = _build_weight_blocks(w_pre, w_post)

    if "nc" not in _CACHE:
        _CACHE["nc"] = _build_nc(block_meta, wall.shape[1])
    nc = _CACHE["nc"]

    in_maps = build_in_maps(x, wall)
    res = run_bass_kernel_spmd(nc, in_maps, core_ids=list(range(N_CORES)))
    yp = np.stack([res.results[b]["ys"] for b in range(N_CORES)])  # [B,g,j,T]
    out = (
        yp.reshape(B, SEG, C, NSEG, T)
        .transpose(0, 2, 4, 3, 1)  # [B, C, T, j, u]
        .reshape(B, C, T, FPAD)[:, :, :, FOFF : FOFF + F]
        .astype(np.float32)
    )

    if np.any(b_pre) or np.any(b_post):
        field = _bias_field(bands, b_pre, w_post, b_post)
        out = out + field[None, :, None, :]
    return np.ascontiguousarray(out)



# revision 2
# speedup vs baseline: 76331.5780x; 76331.5780x over previous
"""BandSplitLinear Trainium2 kernel (host-transposed fp16 I/O, pure matmul).

Strategy (per core, batch-parallel over 8 cores):
  - Fold w_pre @ w_post into one 128x128 matrix per band on the host (no
    nonlinearity between the linears); biases applied host-side.
  - Carve the frequency axis into 33 aligned segments of 32 bins (grid
    f + 22 = 32*j + u). Every band spans <= 2 adjacent segments, so the
    folded weights form a block-TRIDIAGONAL [33x33] structure of 128-col
    blocks (63 nonzero) over the feature layout g = u*4 + c; the "lower"
    off-diagonal blocks are column-trimmed to the straddling band's tail
    (their matmuls accumulate into the partition prefix [0, 4*uhi)).
  - Host passes x already cast to fp16, zero-padded to the segment grid,
    and permuted to [g = u*4+c (128), j(33), T] — exactly the SBUF layout,
    so loads/stores are plain partition-range strided DMAs.
    On-chip data flow is pure: HBM->SBUF loads, fp16 matmuls with fp32
    PSUM accumulation, PSUM->SBUF cast copies, SBUF->HBM stores in the
    same layout. Zero on-chip transposes, gather/scatter, or packing.
    Host permutes/casts the output back to [C, T, F] fp32.
  - DMA-bound (~18.6 MB/core at the ~358 GB/s per-NC HBM limit). One
    dma_start fans across all 16 SDMA engines, so traffic is organized
    as FEW, LARGE, 128-partition transfers with multi-KB contiguous
    per-partition runs (full-T load groups, full-T store groups): this
    keeps every engine at line rate instead of descriptor overhead.
    Loads ride the SP HWDGE ring, wall + stores ride the Act HWDGE
    ring — two independent FIFO rings that share the 16 engines at
    packet granularity, so reads and writes self-balance to the HBM
    roofline. The gpsimd SWDGE path is unused (saves its end-of-kernel
    drain). A dummy 1-col matmul gates PE start until load group 1 is
    resident so the matmul wave runs warm and contiguous.
"""

import numpy as np

import concourse.tile as tile
from concourse import bacc, mybir
from concourse.bass_utils import run_bass_kernel_spmd


# ---- problem constants (hardcoded per spec) ----
B, C, T, F = 8, 4, 1000, 1025
N_CORES = 8
SEG = 32
FOFF = 22  # grid phase: f + FOFF = 32*j + u; band starts align for f >= 490
NSEG = (F - 1 + FOFF) // SEG + 1  # 33
FPAD = NSEG * SEG  # 1056 padded f rows
P = 128
T_CHUNKS = [(0, 512), (512, 488)]  # PSUM bank granularity for matmul/copy
LOAD_GROUPS = [2, 3, 4, 5, 5, 5, 4, 3, 2]  # j-segments per load group (sum 33)
PE_GATE_GROUP = 1  # first matmul waits for this load group
STORE_GROUPS = [1, 4, 4, 5, 5, 5, 4, 3, 2]  # j-segments per store group (sum 33)
N_SYNC_STORES = 2  # this many trailing store groups ride the SP ring
WALL_SPLIT_BLOCKS = [0, 13, 63]  # wall load split points (block indices)

_F32 = mybir.dt.float32
_F16 = mybir.dt.float16


def _build_bands():
    f, interval = 0, 4
    groups = []
    while f < F:
        end = min(f + interval, F)
        groups.append((f, end))
        f = end
        if interval < 32:
            interval += 1
    return groups  # list of (start, end), disjoint, covering [0, F)


def _block_structure():
    """Nonzero (j_out, j_in) block pairs, grouped by j_out (ascending j_in)."""
    bands = _build_bands()
    pairs = set()
    for start, end in bands:
        segs = set(range((start + FOFF) // SEG, (end - 1 + FOFF) // SEG + 1))
        for ji in segs:
            for jo in segs:
                pairs.add((jo, ji))
    jin_lists = [sorted(ji for (jo, ji) in pairs if jo == j) for j in range(NSEG)]
    return bands, jin_lists


def _build_weight_blocks(w_pre, w_post):
    """Host: fold per-band linears and scatter into segment-pair blocks."""
    bands, jin_lists = _block_structure()
    wc = np.einsum(
        "kio,kod->kid", w_pre.astype(np.float64), w_post.astype(np.float64)
    )  # [45, 128, 128], both feature dims indexed by w*4 + c
    blocks = {}
    for k, (start, end) in enumerate(bands):
        fs = np.arange(start, end)
        js = (fs + FOFF) // SEG
        us = (fs + FOFF) % SEG
        for ji in np.unique(js):
            for jo in np.unique(js):
                key = (int(jo), int(ji))
                if key not in blocks:
                    blocks[key] = np.zeros((P, P), dtype=np.float64)
                blk = blocks[key]
                mi = js == ji
                mo = js == jo
                wi = fs[mi] - start
                wo = fs[mo] - start
                for ci in range(C):
                    for co in range(C):
                        # feature layout g = u*4 + c (u-major, c interleaved)
                        blk[np.ix_(us[mi] * C + ci, us[mo] * C + co)] = wc[k][
                            np.ix_(wi * C + ci, wo * C + co)
                        ]
    # Per jo, order blocks [diag, upper (ji=jo+1), lower (ji=jo-1)]. The
    # lower block's nonzero output columns are only g < 4*uhi (the
    # straddling band's tail at the bottom of seg jo) -- trim them: that
    # matmul then writes just partitions [0, 4*uhi), accumulating after the
    # full-width diag/upper matmuls. Saves wall bytes; numerically
    # identical (trimmed columns are exact zeros).
    cols = []
    block_meta = []  # per jo: list of (ji, col_off, ncols)
    off = 0
    for jo in range(NSEG):
        metas = []
        for ji in [jo] + [j for j in (jo + 1, jo - 1) if j in jin_lists[jo]]:
            blk = blocks[(jo, ji)]
            if ji == jo - 1:
                fb = SEG * jo - FOFF
                (uhi,) = [e - fb for (s, e) in bands if s < fb < e]
                blk = blk[:, : C * uhi]
            metas.append((ji, off, blk.shape[1]))
            cols.append(blk)
            off += blk.shape[1]
        block_meta.append(metas)
    wall = np.concatenate(cols, axis=1).astype(np.float16)  # [128, total]
    return wall, block_meta, jin_lists


def _bias_field(bands, b_pre, w_post, b_post):
    """bias[c, f]: the constant added to out[., c, ., f]."""
    bc = (
        np.einsum("ko,kod->kd", b_pre.astype(np.float64), w_post.astype(np.float64))
        + b_post.astype(np.float64)
    )
    field = np.zeros((C, F), dtype=np.float64)
    for k, (start, end) in enumerate(bands):
        for c in range(C):
            field[c, start:end] = bc[k, (np.arange(end - start)) * C + c]
    return field.astype(np.float32)


def _build_nc(block_meta, total_cols):
    nc = bacc.Bacc("TRN2", target_bir_lowering=False, debug=False)
    xs = nc.dram_tensor("xs", [P, NSEG, T], _F16, kind="ExternalInput")
    wall = nc.dram_tensor("wall", [P, total_cols], _F16, kind="ExternalInput")
    ys = nc.dram_tensor("ys", [P, NSEG, T], _F16, kind="ExternalOutput")

    flat = [m for metas in block_meta for m in metas]
    block_coffs = [m[1] for m in flat] + [total_cols]
    wall_ranges = [
        (block_coffs[lo], block_coffs[hi])
        for lo, hi in zip(WALL_SPLIT_BLOCKS, WALL_SPLIT_BLOCKS[1:])
    ]
    # valid partitions: j=0 has g >= 4*FOFF, j=32 has g < 4*UTAIL
    UTAIL = F - (SEG * (NSEG - 1) - FOFF)  # 23 valid u rows in j=32

    with tile.TileContext(nc) as tc:
        import contextlib

        ctx = contextlib.ExitStack()
        with ctx:
            wall_pool = ctx.enter_context(tc.tile_pool(name="wall", bufs=1))
            at_pools = [
                ctx.enter_context(tc.tile_pool(name=f"atg{i}", bufs=1))
                for i in range(len(LOAD_GROUPS))
            ]
            y_pools = [
                ctx.enter_context(tc.tile_pool(name=f"yg{i}", bufs=1))
                for i in range(len(STORE_GROUPS))
            ]
            psy_pool = ctx.enter_context(
                tc.tile_pool(name="psy", bufs=8, space="PSUM")
            )

            # ---- input tiles: [g = u*4+c, j*T + t] per group ----
            at_tiles = []  # (j0, tile) per group
            j0 = 0
            for gi, gn in enumerate(LOAD_GROUPS):
                at_tiles.append(
                    (j0, at_pools[gi].tile([P, gn * T], _F16, name=f"atg{gi}"))
                )
                j0 += gn

            wall_sb = wall_pool.tile([P, total_cols], _F16)

            # Wall rides the Act ring ahead of any store; the first range
            # covers the low-jo blocks so the matmul wave can start as
            # soon as load group 0 lands.
            for lo, hi in wall_ranges:
                nc.scalar.dma_start(wall_sb[:, lo:hi], wall.ap()[:, lo:hi])

            # Input loads: one full-width, full-T transfer per group on
            # the SP ring. Per-partition runs are gn*2000 B contiguous.
            for gi, gn in enumerate(LOAD_GROUPS):
                j0, at_t = at_tiles[gi]
                nc.sync.dma_start(
                    at_t.rearrange("p (j t) -> p j t", j=gn),
                    xs.ap()[:, j0 : j0 + gn, :],
                )

            def at_slice(ji, t0, tn):
                for gi, gn in enumerate(LOAD_GROUPS):
                    j0, at_t = at_tiles[gi]
                    if j0 <= ji < j0 + gn:
                        return at_t[:, (ji - j0) * T + t0 : (ji - j0) * T + t0 + tn]
                raise AssertionError(ji)

            # ---- y staging tiles per store group ----
            y_tiles = []
            j0 = 0
            for gi, gn in enumerate(STORE_GROUPS):
                y_tiles.append(
                    (j0, y_pools[gi].tile([P, gn * T], _F16, name=f"yg{gi}"))
                )
                j0 += gn

            # ---- matmul wavefront over jo, PSUM -> y copies, group stores ----
            # PE gate: a 1-column dummy matmul on a later load group delays
            # PE's in-order stream until enough input is resident, so the
            # real matmuls run back-to-back at full (warm) clock instead of
            # chasing the load wavefront through p-state resets.
            gate_ps = psy_pool.tile([P, 512], _F32, name="psy")
            nc.tensor.matmul(
                gate_ps[:, 0:1],
                lhsT=wall_sb[:, 0:P],
                rhs=at_tiles[PE_GATE_GROUP][1][:, 0:1],
                start=True,
                stop=True,
            )

            gi_store = 0
            for jo in range(NSEG):
                metas = block_meta[jo]
                nw = len(metas)
                yj0, y_t = y_tiles[gi_store]
                for t0, tn in T_CHUNKS:
                    psy = psy_pool.tile([P, 512], _F32, name="psy")
                    for i, (ji, coff, ncols) in enumerate(metas):
                        # trimmed lower blocks write only partitions
                        # [0, ncols), accumulating onto the full-width result
                        nc.tensor.matmul(
                            psy[0:ncols, 0:tn],
                            lhsT=wall_sb[:, coff : coff + ncols],
                            rhs=at_slice(ji, t0, tn),
                            start=(i == 0),
                            stop=(i == nw - 1),
                        )
                    dst = y_t[:, (jo - yj0) * T + t0 : (jo - yj0) * T + t0 + tn]
                    if jo % 2 == 0:
                        nc.scalar.copy(dst, psy[:, 0:tn])
                    else:
                        nc.vector.tensor_copy(dst, psy[:, 0:tn])

                # group finished -> store it as one full-width transfer.
                # Pad partitions of the first/last segment carry exact
                # zeros (zero wall columns), but we trim them anyway since
                # they are the group's only segment.
                gn = STORE_GROUPS[gi_store]
                if jo == yj0 + gn - 1:
                    eng = (
                        nc.sync
                        if gi_store >= len(STORE_GROUPS) - N_SYNC_STORES
                        else nc.scalar
                    )
                    if gi_store == 0:
                        # group [j=0]: only g >= 4*FOFF valid
                        eng.dma_start(
                            ys.ap()[C * FOFF :, 0, :], y_t[C * FOFF :, 0:T]
                        )
                    elif gi_store == len(STORE_GROUPS) - 1:
                        # last group [.., j=32]: j=32 only g < 4*UTAIL valid
                        src = y_t.rearrange("p (j t) -> p j t", j=gn)
                        if gn > 1:
                            eng.dma_start(
                                ys.ap()[:, yj0 : yj0 + gn - 1, :],
                                src[:, 0 : gn - 1, :],
                            )
                        eng.dma_start(
                            ys.ap()[0 : C * UTAIL, NSEG - 1, :],
                            y_t[0 : C * UTAIL, (gn - 1) * T : gn * T],
                        )
                    else:
                        eng.dma_start(
                            ys.ap()[:, yj0 : yj0 + gn, :],
                            y_t.rearrange("p (j t) -> p j t", j=gn),
                        )
                    gi_store += 1
    nc.compile()
    return nc


_CACHE = {}


def build_in_maps(x, wall):
    """Host prep: wall is already the flat [g_in, packed cols] matrix; x is
    cast fp16, padded to the 1056-row segment grid, permuted to
    [g = u*4+c (128), j(33), T] so each SBUF partition reads one contiguous
    DRAM run."""
    wall2 = np.ascontiguousarray(wall)
    xp = np.zeros((B, C, FPAD, T), dtype=np.float16)
    xp[:, :, FOFF : FOFF + F, :] = np.asarray(x, np.float32).astype(
        np.float16
    ).transpose(0, 1, 3, 2)
    xp = np.ascontiguousarray(
        xp.reshape(B, C, NSEG, SEG, T)
        .transpose(0, 3, 1, 2, 4)  # [B, u, c, j, T]
        .reshape(B, P, NSEG, T)
    )
    return [{"xs": xp[b], "wall": wall2} for b in range(N_CORES)]


def kernel(x, w_pre, b_pre, w_post, b_post):
    x = np.asarray(x, dtype=np.float32)
    w_pre = np.asarray(w_pre, dtype=np.float32)
    b_pre = np.asarray(b_pre, dtype=np.float32)
    w_post = np.asarray(w_post, dtype=np.float32)
    b_post = np.asarray(b_post, dtype=np.float32)

    bands, _ = _block_structure()
    wall, block_meta, _ = _build_weight_blocks(w_pre, w_post)

    if "nc" not in _CACHE:
        _CACHE["nc"] = _build_nc(block_meta, wall.shape[1])
    nc = _CACHE["nc"]

    in_maps = build_in_maps(x, wall)
    res = run_bass_kernel_spmd(nc, in_maps, core_ids=list(range(N_CORES)))
    yp = np.stack([res.results[b]["ys"] for b in range(N_CORES)])  # [B,g,j,T]
    out = (
        yp.reshape(B, SEG, C, NSEG, T)
        .transpose(0, 2, 4, 3, 1)  # [B, C, T, j, u]
        .reshape(B, C, T, FPAD)[:, :, :, FOFF : FOFF + F]
        .astype(np.float32)
    )

    if np.any(b_pre) or np.any(b_post):
        field = _bias_field(bands, b_pre, w_post, b_post)
        out = out + field[None, :, None, :]
    return np.ascontiguousarray(out)


# revision 4
# speedup vs baseline: 80244.7164x; 1.0513x over previous
"""BandSplitLinear Trainium2 kernel (host-transposed fp16 I/O, pure matmul).

Strategy (per core, batch-parallel over 8 cores):
  - Fold w_pre @ w_post into one 128x128 matrix per band on the host (no
    nonlinearity between the linears); biases applied host-side.
  - Carve the frequency axis into 33 aligned segments of 32 bins (grid
    f + 22 = 32*j + u). Every band spans <= 2 adjacent segments, so the
    folded weights form a block-TRIDIAGONAL [33x33] structure of 128-col
    blocks (63 nonzero) over the feature layout g = u*4 + c; the "lower"
    off-diagonal blocks are column-trimmed to the straddling band's tail
    (their matmuls accumulate into the partition prefix [0, 4*uhi)).
  - Host passes x already cast to fp16, zero-padded to the segment grid,
    and permuted to [g = u*4+c (128), j(33), T] — exactly the SBUF layout,
    so loads/stores are plain partition-range strided DMAs.
    On-chip data flow is pure: HBM->SBUF loads, fp16 matmuls with fp32
    PSUM accumulation, PSUM->SBUF cast copies, SBUF->HBM stores in the
    same layout. Zero on-chip transposes, gather/scatter, or packing.
    Host permutes/casts the output back to [C, T, F] fp32.
  - DMA-bound (~18.6 MB/core at the ~358 GB/s per-NC HBM limit). One
    dma_start fans across all 16 SDMA engines, so traffic is organized
    as FEW, LARGE, 128-partition transfers with multi-KB contiguous
    per-partition runs (full-T load groups, full-T store groups): this
    keeps every engine at line rate instead of descriptor overhead.
    Loads ride the SP HWDGE ring, wall + stores ride the Act HWDGE
    ring — two independent FIFO rings that share the 16 engines at
    packet granularity, so reads and writes self-balance to the HBM
    roofline. The gpsimd SWDGE path is unused (saves its end-of-kernel
    drain). A dummy 1-col matmul gates PE start until load group 1 is
    resident so the matmul wave runs warm and contiguous.
"""

import numpy as np

import concourse.tile as tile
from concourse import bacc, mybir
from concourse.bass_utils import run_bass_kernel_spmd


# ---- problem constants (hardcoded per spec) ----
B, C, T, F = 8, 4, 1000, 1025
N_CORES = 8
SEG = 32
FOFF = 22  # grid phase: f + FOFF = 32*j + u; band starts align for f >= 490
NSEG = (F - 1 + FOFF) // SEG + 1  # 33
FPAD = NSEG * SEG  # 1056 padded f rows
P = 128
T_CHUNKS = [(0, 512), (512, 488)]  # PSUM bank granularity for matmul/copy
LOAD_GROUPS = [2, 2, 3, 4, 4, 5, 5, 4, 2, 2]  # j-segments per load group (sum 33)
PE_GATE_GROUP = 1  # first matmul waits for this load group
STORE_GROUPS = [1, 2, 3, 3, 4, 4, 4, 4, 3, 2, 2, 1]  # j-segs per store group (33)
N_SYNC_STORES = 2  # this many trailing store groups ride the SP ring
WALL_SPLIT_JO = [0, 2, 14, NSEG]  # wall load split points (jo boundaries)

_F32 = mybir.dt.float32
_F16 = mybir.dt.float16


def _build_bands():
    f, interval = 0, 4
    groups = []
    while f < F:
        end = min(f + interval, F)
        groups.append((f, end))
        f = end
        if interval < 32:
            interval += 1
    return groups  # list of (start, end), disjoint, covering [0, F)


def _block_structure():
    """Nonzero (j_out, j_in) block pairs, grouped by j_out (ascending j_in)."""
    bands = _build_bands()
    pairs = set()
    for start, end in bands:
        segs = set(range((start + FOFF) // SEG, (end - 1 + FOFF) // SEG + 1))
        for ji in segs:
            for jo in segs:
                pairs.add((jo, ji))
    jin_lists = [sorted(ji for (jo, ji) in pairs if jo == j) for j in range(NSEG)]
    return bands, jin_lists


def _build_weight_blocks(w_pre, w_post):
    """Host: fold per-band linears and scatter into segment-pair blocks."""
    bands, jin_lists = _block_structure()
    wc = np.einsum(
        "kio,kod->kid", w_pre.astype(np.float64), w_post.astype(np.float64)
    )  # [45, 128, 128], both feature dims indexed by w*4 + c
    blocks = {}
    for k, (start, end) in enumerate(bands):
        fs = np.arange(start, end)
        js = (fs + FOFF) // SEG
        us = (fs + FOFF) % SEG
        for ji in np.unique(js):
            for jo in np.unique(js):
                key = (int(jo), int(ji))
                if key not in blocks:
                    blocks[key] = np.zeros((P, P), dtype=np.float64)
                blk = blocks[key]
                mi = js == ji
                mo = js == jo
                wi = fs[mi] - start
                wo = fs[mo] - start
                for ci in range(C):
                    for co in range(C):
                        # feature layout g = u*4 + c (u-major, c interleaved)
                        blk[np.ix_(us[mi] * C + ci, us[mo] * C + co)] = wc[k][
                            np.ix_(wi * C + ci, wo * C + co)
                        ]
    # Per jo, order blocks [diag, upper (ji=jo+1), lower (ji=jo-1)]. The
    # lower block's nonzero output columns are only g < 4*uhi (the
    # straddling band's tail at the bottom of seg jo) -- trim them: that
    # matmul then writes just partitions [0, 4*uhi), accumulating after the
    # full-width diag/upper matmuls. Saves wall bytes; numerically
    # identical (trimmed columns are exact zeros).
    cols = []
    block_meta = []  # per jo: list of (ji, col_off, ncols)
    off = 0
    for jo in range(NSEG):
        metas = []
        for ji in [jo] + [j for j in (jo + 1, jo - 1) if j in jin_lists[jo]]:
            blk = blocks[(jo, ji)]
            if ji == jo - 1:
                fb = SEG * jo - FOFF
                (uhi,) = [e - fb for (s, e) in bands if s < fb < e]
                blk = blk[:, : C * uhi]
            metas.append((ji, off, blk.shape[1]))
            cols.append(blk)
            off += blk.shape[1]
        block_meta.append(metas)
    wall = np.concatenate(cols, axis=1).astype(np.float16)  # [128, total]
    return wall, block_meta, jin_lists


def _bias_field(bands, b_pre, w_post, b_post):
    """bias[c, f]: the constant added to out[., c, ., f]."""
    bc = (
        np.einsum("ko,kod->kd", b_pre.astype(np.float64), w_post.astype(np.float64))
        + b_post.astype(np.float64)
    )
    field = np.zeros((C, F), dtype=np.float64)
    for k, (start, end) in enumerate(bands):
        for c in range(C):
            field[c, start:end] = bc[k, (np.arange(end - start)) * C + c]
    return field.astype(np.float32)


def _build_nc(block_meta, total_cols):
    nc = bacc.Bacc("TRN2", target_bir_lowering=False, debug=False)
    xs = nc.dram_tensor("xs", [P, NSEG, T], _F16, kind="ExternalInput")
    wall = nc.dram_tensor("wall", [P, total_cols], _F16, kind="ExternalInput")
    ys = nc.dram_tensor("ys", [P, NSEG, T], _F16, kind="ExternalOutput")

    # wall ranges by jo boundary: range i covers all blocks of
    # jo in [WALL_SPLIT_JO[i], WALL_SPLIT_JO[i+1])
    jo_coff = [block_meta[jo][0][1] for jo in range(NSEG)] + [total_cols]
    wall_ranges = [
        (jo_coff[lo], jo_coff[hi])
        for lo, hi in zip(WALL_SPLIT_JO, WALL_SPLIT_JO[1:])
    ]
    # valid partitions: j=0 has g >= 4*FOFF, j=32 has g < 4*UTAIL
    UTAIL = F - (SEG * (NSEG - 1) - FOFF)  # 23 valid u rows in j=32

    with tile.TileContext(nc) as tc:
        import contextlib

        ctx = contextlib.ExitStack()
        with ctx:
            wall_pool = ctx.enter_context(tc.tile_pool(name="wall", bufs=1))
            at_pools = [
                ctx.enter_context(tc.tile_pool(name=f"atg{i}", bufs=1))
                for i in range(len(LOAD_GROUPS))
            ]
            y_pools = [
                ctx.enter_context(tc.tile_pool(name=f"yg{i}", bufs=1))
                for i in range(len(STORE_GROUPS))
            ]
            psy_pool = ctx.enter_context(
                tc.tile_pool(name="psy", bufs=8, space="PSUM")
            )

            # ---- input tiles: [g = u*4+c, j*T + t] per group ----
            at_tiles = []  # (j0, tile) per group
            j0 = 0
            for gi, gn in enumerate(LOAD_GROUPS):
                at_tiles.append(
                    (j0, at_pools[gi].tile([P, gn * T], _F16, name=f"atg{gi}"))
                )
                j0 += gn

            wall_sb = wall_pool.tile([P, total_cols], _F16)

            # Wall rides the Act ring ahead of any store; the first range
            # covers the low-jo blocks so the matmul wave can start as
            # soon as load group 0 lands.
            for lo, hi in wall_ranges:
                nc.scalar.dma_start(wall_sb[:, lo:hi], wall.ap()[:, lo:hi])

            # Input loads: one full-width, full-T transfer per group on
            # the SP ring. Per-partition runs are gn*2000 B contiguous.
            for gi, gn in enumerate(LOAD_GROUPS):
                j0, at_t = at_tiles[gi]
                nc.sync.dma_start(
                    at_t.rearrange("p (j t) -> p j t", j=gn),
                    xs.ap()[:, j0 : j0 + gn, :],
                )

            def at_slice(ji, t0, tn):
                for gi, gn in enumerate(LOAD_GROUPS):
                    j0, at_t = at_tiles[gi]
                    if j0 <= ji < j0 + gn:
                        return at_t[:, (ji - j0) * T + t0 : (ji - j0) * T + t0 + tn]
                raise AssertionError(ji)

            # ---- y staging tiles per store group ----
            y_tiles = []
            j0 = 0
            for gi, gn in enumerate(STORE_GROUPS):
                y_tiles.append(
                    (j0, y_pools[gi].tile([P, gn * T], _F16, name=f"yg{gi}"))
                )
                j0 += gn

            # ---- matmul wavefront over jo, PSUM -> y copies, group stores ----
            # PE gate: a 1-column dummy matmul on a later load group delays
            # PE's in-order stream until enough input is resident, so the
            # real matmuls run back-to-back at full (warm) clock instead of
            # chasing the load wavefront through p-state resets.
            gate_ps = psy_pool.tile([P, 512], _F32, name="psy")
            nc.tensor.matmul(
                gate_ps[:, 0:1],
                lhsT=wall_sb[:, 0:P],
                rhs=at_tiles[PE_GATE_GROUP][1][:, 0:1],
                start=True,
                stop=True,
            )

            gi_store = 0
            for jo in range(NSEG):
                metas = block_meta[jo]
                nw = len(metas)
                yj0, y_t = y_tiles[gi_store]
                for t0, tn in T_CHUNKS:
                    psy = psy_pool.tile([P, 512], _F32, name="psy")
                    for i, (ji, coff, ncols) in enumerate(metas):
                        # trimmed lower blocks write only partitions
                        # [0, ncols), accumulating onto the full-width result
                        nc.tensor.matmul(
                            psy[0:ncols, 0:tn],
                            lhsT=wall_sb[:, coff : coff + ncols],
                            rhs=at_slice(ji, t0, tn),
                            start=(i == 0),
                            stop=(i == nw - 1),
                        )
                    dst = y_t[:, (jo - yj0) * T + t0 : (jo - yj0) * T + t0 + tn]
                    if jo % 2 == 0:
                        nc.scalar.copy(dst, psy[:, 0:tn])
                    else:
                        nc.vector.tensor_copy(dst, psy[:, 0:tn])

                # group finished -> store it as one full-width transfer.
                # Pad partitions of the first/last segment carry exact
                # zeros (zero wall columns), but we trim them anyway since
                # they are the group's only segment.
                gn = STORE_GROUPS[gi_store]
                if jo == yj0 + gn - 1:
                    eng = (
                        nc.sync
                        if gi_store >= len(STORE_GROUPS) - N_SYNC_STORES
                        else nc.scalar
                    )
                    if gi_store == 0:
                        # group [j=0]: only g >= 4*FOFF valid
                        eng.dma_start(
                            ys.ap()[C * FOFF :, 0, :], y_t[C * FOFF :, 0:T]
                        )
                    elif gi_store == len(STORE_GROUPS) - 1:
                        # last group [.., j=32]: j=32 only g < 4*UTAIL valid
                        src = y_t.rearrange("p (j t) -> p j t", j=gn)
                        if gn > 1:
                            eng.dma_start(
                                ys.ap()[:, yj0 : yj0 + gn - 1, :],
                                src[:, 0 : gn - 1, :],
                            )
                        eng.dma_start(
                            ys.ap()[0 : C * UTAIL, NSEG - 1, :],
                            y_t[0 : C * UTAIL, (gn - 1) * T : gn * T],
                        )
                    else:
                        eng.dma_start(
                            ys.ap()[:, yj0 : yj0 + gn, :],
                            y_t.rearrange("p (j t) -> p j t", j=gn),
                        )
                    gi_store += 1
    nc.compile()
    return nc


_CACHE = {}


def build_in_maps(x, wall):
    """Host prep: wall is already the flat [g_in, packed cols] matrix; x is
    cast fp16, padded to the 1056-row segment grid, permuted to
    [g = u*4+c (128), j(33), T] so each SBUF partition reads one contiguous
    DRAM run."""
    wall2 = np.ascontiguousarray(wall)
    xp = np.zeros((B, C, FPAD, T), dtype=np.float16)
    xp[:, :, FOFF : FOFF + F, :] = np.asarray(x, np.float32).astype(
        np.float16
    ).transpose(0, 1, 3, 2)
    xp = np.ascontiguousarray(
        xp.reshape(B, C, NSEG, SEG, T)
        .transpose(0, 3, 1, 2, 4)  # [B, u, c, j, T]
        .reshape(B, P, NSEG, T)
    )
    return [{"xs": xp[b], "wall": wall2} for b in range(N_CORES)]


def kernel(x, w_pre, b_pre, w_post, b_post):
    x = np.asarray(x, dtype=np.float32)
    w_pre = np.asarray(w_pre, dtype=np.float32)
    b_pre = np.asarray(b_pre, dtype=np.float32)
    w_post = np.asarray(w_post, dtype=np.float32)
    b_post = np.asarray(b_post, dtype=np.float32)

    bands, _ = _block_structure()
    wall, block_meta, _ = _build_weight_blocks(w_pre, w_post)

    if "nc" not in _CACHE:
        _CACHE["nc"] = _build_nc(block_meta, wall.shape[1])
    nc = _CACHE["nc"]

    in_maps = build_in_maps(x, wall)
    res = run_bass_kernel_spmd(nc, in_maps, core_ids=list(range(N_CORES)))
    yp = np.stack([res.results[b]["ys"] for b in range(N_CORES)])  # [B,g,j,T]
    out = (
        yp.reshape(B, SEG, C, NSEG, T)
        .transpose(0, 2, 4, 3, 1)  # [B, C, T, j, u]
        .reshape(B, C, T, FPAD)[:, :, :, FOFF : FOFF + F]
        .astype(np.float32)
    )

    if np.any(b_pre) or np.any(b_post):
        field = _bias_field(bands, b_pre, w_post, b_post)
        out = out + field[None, :, None, :]
    return np.ascontiguousarray(out)
